# revision 1
# baseline (speedup 1.0000x reference)
import sys

sys.path.insert(0, "/opt/trn_rl_repo")

import numpy as np
import ml_dtypes

import concourse.mybir as mybir
from concourse import bass, tile
from concourse import tile_sem_assignment as _tsa
from concourse.bass_utils import run_bass_kernel_spmd
from concourse.vector_clock import ScopedClock, VectorClock

_orig_drain_and_barrier = tile.TileContext._drain_and_barrier


def _split_drain_and_barrier(self, tick_clock, wait_clock):
    # The final Drain waits on every active semaphore at once; with 8 HWDGE
    # lanes + SWDGE + 3 engines that exceeds the CTRL instruction's sync
    # wait slots. Emit one 1-wait drain per proc instead (same semantics:
    # SP executes them in order, so all sems reach their targets before the
    # barrier), then replicate the original barrier/cleanup sequence.
    gc = tick_clock.global_clock
    n = _tsa.N_PROCS
    for p in range(n):
        if gc[p] > 0:
            partial = VectorClock([gc[q] if q == p else 0 for q in range(n)])
            d = self.nc.sync.drain()
            wait_clock.add_sem_waits(d.ins, ScopedClock({None: partial}))
    self.nc.all_engine_barrier()
    popped = self.nc._tile_sem_poison_stack.pop()
    assert popped is self._sem_poison
    self.nc.clear_and_free_semaphores(list(self.sems.allocated().values()))
    self.nc.all_engine_barrier()


tile.TileContext._drain_and_barrier = _split_drain_and_barrier

B = 1024        # batch rows of address
N = 65536       # mem rows (sharded)
M = 128         # mem cols
NCORES = 8
NS = N // NCORES          # 8192 rows per core
NCHUNKS = NS // 128       # 64 chunks of 128 mem-rows
MCHUNKS = NS // 256       # 32 mega-chunks of 256 mem-rows (DoubleRow)
BCHUNKS = B // 128        # 8 chunks of 128 batch-rows
NSTAGES = 8               # DMA pipeline stages (8 chunks each)

FP8 = mybir.dt.float8e4
BF16 = mybir.dt.bfloat16
F32 = mybir.dt.float32
DR = mybir.MatmulPerfMode.DoubleRow
ADD = mybir.AluOpType.add
MULT = mybir.AluOpType.mult

_compiled = {}


NG = 16  # DMA groups; each covers 4 mem-chunks (k) = 2 mega-chunks (ch)


def _build_nc():
    nc = bass.Bass(target_bir_lowering=False)

    # a:  [p=b%128, j(n-slice of 1024), ub(u-block), bc, u]  A shard for GEMM1
    a = nc.dram_tensor("a", [128, NSTAGES, 8, BCHUNKS, 128], FP8, kind="ExternalInput")
    # at: [p=n%128 within 256-chunk, ch, sub, b]  A^T shard for GEMM2 (partition=n)
    at = nc.dram_tensor("at", [128, MCHUNKS, 2, B], FP8, kind="ExternalInput")
    # c:  [p=n%128, k, m]  0.5*content shard (partition=n)
    c = nc.dram_tensor("c", [128, NCHUNKS, M], FP8, kind="ExternalInput")
    # ed: [p=b%128, h(hi/lo), bc, 2M]  [-erase | 0.5*add] split as hi+lo fp8
    ed = nc.dram_tensor("ed", [128, 2, BCHUNKS, 2 * M], FP8, kind="ExternalInput")
    # rt: [m, b] partial (read/2)^T fp32
    rt = nc.dram_tensor("rt", [M, B], F32, kind="ExternalOutput")

    with tile.TileContext(nc) as tc:
        with (
            tc.tile_pool(name="abuf", bufs=1) as a_pool,
            tc.tile_pool(name="atbuf", bufs=1) as at_pool,
            tc.tile_pool(name="cbuf", bufs=1) as c_pool,
            tc.tile_pool(name="edbuf", bufs=1) as ed_pool,
            tc.tile_pool(name="tmpbuf", bufs=8) as tmp_pool,
            tc.tile_pool(name="cpbuf", bufs=6) as cp_pool,
            tc.tile_pool(name="rtbuf", bufs=1) as rt_pool,
            tc.tile_pool(name="pw", bufs=6, space="PSUM") as pw_pool,
            tc.tile_pool(name="pr", bufs=1, space="PSUM") as pr_pool,
        ):
            a_t = a_pool.tile([128, NSTAGES, 8, BCHUNKS, 128], FP8)
            at_t = at_pool.tile([128, MCHUNKS, 2, B], FP8)
            c_t = c_pool.tile([128, NCHUNKS, M], FP8)
            ed_t = ed_pool.tile([128, 2, BCHUNKS, 2 * M], FP8)

            # Fine-grained preloads in consumption order so DMA arrival
            # tracks compute need and the scheduler interleaves G1/G2
            # naturally (the wait-dedup chain needs G2 close behind G1).
            # Preload DMAs write each SBUF dest exactly once, so their only
            # wait is the HWDGE lane-credit wait (1 wait, allowed). The rt
            # store goes out over SWDGE (gpsimd) so it lands on a fresh
            # lane and carries only its RAW wait.
            def a_group(g):
                j, ub0 = g // 2, (g % 2) * 4
                nc.sync.dma_start(
                    out=a_t[:, j, ub0 : ub0 + 4], in_=a[:, j, ub0 : ub0 + 4]
                )

            # 'a' leads 'at' by one group so the G1->STT->TADD chain for the
            # final chunks drains while the last at groups are still in
            # flight; the last at group is split per mega-chunk so only
            # G2(31) + copy/store trail the final DMA.
            nc.sync.dma_start(out=ed_t[:], in_=ed[:])
            nc.sync.dma_start(out=c_t[:, 0:32, :], in_=c[:, 0:32, :])
            a_group(0)
            for g in range(NG - 2):
                a_group(g + 1)
                if g == 7:
                    nc.sync.dma_start(out=c_t[:, 32:64, :], in_=c[:, 32:64, :])
                nc.sync.dma_start(
                    out=at_t[:, 2 * g : 2 * g + 2], in_=at[:, 2 * g : 2 * g + 2]
                )
            a_group(NG - 1)
            nc.sync.dma_start(out=at_t[:, 28:30], in_=at[:, 28:30])
            nc.sync.dma_start(out=at_t[:, 30:31], in_=at[:, 30:31])
            nc.sync.dma_start(out=at_t[:, 31:32], in_=at[:, 31:32])

            psum_r = pr_pool.tile([128, B], F32)
            land = tmp_pool.tile([128, 1], F32)
            # Wake the Activation engine early: its first instruction carries
            # a ~1.4us startup cost in the model; pay it off the critical
            # path so the tail copies run at steady-state rate.
            warm = tmp_pool.tile([128, 1], F32)
            nc.scalar.copy(warm[:], ed_t[:, 0, 0, 0:1])

            def emit_g2(ch, cp):
                for jj in range(2):
                    nc.tensor.matmul(
                        psum_r[:, jj * 512 : (jj + 1) * 512],
                        cp[:],
                        at_t[:, ch, :, jj * 512 : (jj + 1) * 512],
                        start=(ch == 0),
                        stop=(ch == MCHUNKS - 1),
                        perf_mode=DR,
                    )

            cp = None
            for k in range(NCHUNKS):
                ch, sub = k // 2, k % 2
                j, ub = k // 8, k % 8
                if k % 32 == 0:
                    # DVE absorbs this c-half's DMA wait so STT(k) keeps
                    # only its PSUM-read wait (dedup on the same lane sem).
                    nc.vector.tensor_copy(land[:], c_t[:, k, 0:1])

                if sub == 0:
                    cp = cp_pool.tile([128, 2, M], FP8)

                psum_w = pw_pool.tile([128, 2 * M], F32)
                for h in range(2):
                    for q in range(4):
                        nc.tensor.matmul(
                            psum_w[:],
                            a_t[:, j, ub, 2 * q : 2 * q + 2, :],
                            ed_t[:, h, 2 * q : 2 * q + 2, :],
                            start=(h == 0 and q == 0),
                            stop=(h == 1 and q == 3),
                            perf_mode=DR,
                        )

                # psum_w = [-We | Wa/2];  C'/2 = (1 - We) * (C/2) + Wa/2
                tmp2 = tmp_pool.tile([128, M], F32)
                nc.vector.scalar_tensor_tensor(
                    tmp2[:], psum_w[:, 0:M], 1.0, c_t[:, k, :], ADD, MULT
                )
                nc.vector.tensor_add(cp[:, sub, :], tmp2[:], psum_w[:, M : 2 * M])

                if sub == 1:
                    # G2 Ldweights (stationary=cp) carries DVE>=tadd(2ch+1),
                    # covering the bank-WAR waits of later G1 start-matmuls
                    # via per-engine wait dedup.
                    emit_g2(ch, cp)

            # Split the tail: psum_r bank jj completes at G2(ch=31, jj), so
            # copy+store each half as soon as its accumulation stops instead
            # of one serial full-width copy followed by one big store.
            rt_t = rt_pool.tile([128, B], F32)
            for jj in range(2):
                nc.scalar.copy(
                    rt_t[:, jj * 512 : (jj + 1) * 512],
                    psum_r[:, jj * 512 : (jj + 1) * 512],
                )
                nc.gpsimd.dma_start(
                    out=rt[:, jj * 512 : (jj + 1) * 512],
                    in_=rt_t[:, jj * 512 : (jj + 1) * 512],
                )

    # The scheduler can hoist a G1 start-Matmult ahead of the G2 Ldweights
    # whose DVE wait would dedup-cover its bank-WAR wait, leaving it with
    # two waits (PE self-wait + DVE) — one over the HW wait-slot limit.
    # The same-engine self-wait is always satisfied by in-order queue
    # completion, so drop it.
    for inst in nc.inst_map.values():
        si = inst.sync_info
        if si and si.on_wait and len(si.on_wait) > 1:
            eng = str(inst.engine).split(".")[-1]
            kept = [w for w in si.on_wait if not w.ant_name.startswith(eng + "_")]
            assert len(kept) == 1
            si.on_wait = kept

    return nc


def _prep_inputs(address, erase, add, content):
    f8 = ml_dtypes.float8_e4m3
    a_f8 = address.astype(f8)                                 # [1024, 65536]
    ed = np.concatenate([-erase, 0.5 * add], axis=1)          # [1024, 256] f32
    ed_hi = ed.astype(f8)
    ed_lo = (ed - ed_hi.astype(np.float32)).astype(f8)
    ed_st = np.stack([ed_hi, ed_lo])                          # [2, 1024, 256]
    ed_r = np.ascontiguousarray(
        ed_st.reshape(2, BCHUNKS, 128, 2 * M).transpose(2, 0, 1, 3)
    )                                                         # [128, 2, 8, 256]
    c_bf = (0.5 * content).astype(f8)                         # [65536, 128]

    in_maps = []
    for ci in range(NCORES):
        a_c = a_f8[:, ci * NS : (ci + 1) * NS]                # [1024, 8192]
        # a_r[p, j, ub, bc, u] = A[bc*128+p, j*1024+ub*128+u]
        a_r = np.ascontiguousarray(
            a_c.reshape(BCHUNKS, 128, NSTAGES, 8, 128).transpose(1, 2, 3, 0, 4)
        )                                                     # [128, 8, 8, 8, 128]
        # at_r[p, ch, s, b] = A[b, ch*256 + s*128 + p]
        at_r = np.ascontiguousarray(
            a_c.T.reshape(MCHUNKS, 2, 128, B).transpose(2, 0, 1, 3)
        )                                                     # [128, 32, 2, 1024]
        c_c = c_bf[ci * NS : (ci + 1) * NS, :]
        c_r = np.ascontiguousarray(
            c_c.reshape(NCHUNKS, 128, M).transpose(1, 0, 2)
        )                                                     # [128, 64, 128]
        in_maps.append({"a": a_r, "at": at_r, "c": c_r, "ed": ed_r})
    return in_maps


def kernel(address, erase, add, content, _trace=False, _result_box=None):
    if "nc" not in _compiled:
        _compiled["nc"] = _build_nc()
    nc = _compiled["nc"]

    in_maps = _prep_inputs(address, erase, add, content)
    res = run_bass_kernel_spmd(
        nc, in_maps, core_ids=list(range(NCORES)), trace=_trace
    )
    if _result_box is not None:
        _result_box.append(res)

    acc = np.zeros((M, B), dtype=np.float32)
    for r in res.results:
        acc += np.asarray(r["rt"], dtype=np.float32)
    return np.ascontiguousarray((2.0 * acc).T)



# revision 55
# speedup vs baseline: 1.7168x; 1.7168x over previous
import sys

sys.path.insert(0, "/opt/trn_rl_repo")

import numpy as np
import ml_dtypes

import concourse.mybir as mybir
from concourse import bass, tile
from concourse import tile_sem_assignment as _tsa
from concourse.bass_utils import run_bass_kernel_spmd
from concourse.vector_clock import ScopedClock, VectorClock

_orig_drain_and_barrier = tile.TileContext._drain_and_barrier


def _split_drain_and_barrier(self, tick_clock, wait_clock):
    # The final Drain waits on every active semaphore at once; that exceeds
    # the CTRL instruction's sync wait slots. Emit one 1-wait drain per proc
    # instead (same semantics: SP executes them in order, so all sems reach
    # their targets before the barrier), then replicate the original
    # barrier/cleanup sequence.
    gc = tick_clock.global_clock
    n = _tsa.N_PROCS
    for p in range(n):
        if gc[p] > 0:
            partial = VectorClock([gc[q] if q == p else 0 for q in range(n)])
            d = self.nc.sync.drain()
            wait_clock.add_sem_waits(d.ins, ScopedClock({None: partial}))
    self.nc.all_engine_barrier()
    popped = self.nc._tile_sem_poison_stack.pop()
    assert popped is self._sem_poison
    self.nc.clear_and_free_semaphores(list(self.sems.allocated().values()))
    self.nc.all_engine_barrier()


tile.TileContext._drain_and_barrier = _split_drain_and_barrier

B = 1024        # batch rows of address
N = 65536       # mem rows (sharded)
M = 128         # mem cols
NCORES = 8
NS = N // NCORES          # 8192 rows per core
MCHUNKS = NS // 256       # 32 mega-chunks of 256 mem-rows (DoubleRow)
BCHUNKS = B // 128        # 8 chunks of 128 batch-rows
NSTAGES = 8               # a-tensor j groups
KCH = NS // 128           # 64 k-chunks of 128 mem-rows

FP8 = mybir.dt.float8e4
BF16 = mybir.dt.bfloat16
F32 = mybir.dt.float32
DR = mybir.MatmulPerfMode.DoubleRow
ADD = mybir.AluOpType.add
MULT = mybir.AluOpType.mult

_compiled = {}


def _build_nc():
    nc = bass.Bass(target_bir_lowering=False)

    # The three DMA queues (SP-HWDGE, ACT-HWDGE, Pool-SWDGE) are
    # independent ~332GB/s channels in the cost model, so A ships in BOTH
    # layouts (natural for G1's stationary, transposed for G2's moving
    # operand) split across them; no on-chip transpose at all.
    # a:  [p=b%128, j, ub, bc, u]  A shard natural layout (G1 stationary)
    a = nc.dram_tensor("a", [128, NSTAGES, 8, BCHUNKS, 128], FP8, kind="ExternalInput")
    # ats: [p, ch, s, b]  pre-transposed A^T for all chunks (G2 moving)
    ats = nc.dram_tensor("ats", [128, MCHUNKS, 2, B], FP8, kind="ExternalInput")
    # c:  [p=n%128, k, m]  0.5*content shard (partition=n)
    c = nc.dram_tensor("c", [128, KCH, M], FP8, kind="ExternalInput")
    # edid: [p=b%128, 0:8 = [-erase | 0.5*add] bc-chunks, 8:10 = DR identity
    # (only its i=0 diagonal is used, as the +1 const stationary), 10:12 =
    # [ones | zeros] mask. The const matmul adds +1 to the erase half of
    # psum_w so it holds (1 - We) directly (Pool cannot run
    # scalar_tensor_tensor, only plain TensorTensor ops).
    edid = nc.dram_tensor(
        "edid", [128, BCHUNKS + 4, 2 * M], FP8, kind="ExternalInput"
    )
    # rt: [m, b] partial (read/2)^T bf16 (host sums in f32)
    rt = nc.dram_tensor("rt", [M, B], BF16, kind="ExternalOutput")

    with tile.TileContext(nc) as tc:
        with (
            tc.tile_pool(name="abuf", bufs=1) as a_pool,
            tc.tile_pool(name="atbuf", bufs=1) as at_pool,
            tc.tile_pool(name="cbuf", bufs=1) as c_pool,
            tc.tile_pool(name="edbuf", bufs=1) as ed_pool,
            tc.tile_pool(name="tmpbuf", bufs=4) as tmp_pool,
            tc.tile_pool(name="pwsbufa", bufs=3) as pwsa_pool,
            tc.tile_pool(name="pwsbufd", bufs=3) as pwsd_pool,
            tc.tile_pool(name="cpbuf", bufs=8) as cp_pool,
            tc.tile_pool(name="rtbuf", bufs=2) as rt_pool,
            tc.tile_pool(name="landbuf", bufs=6) as land_pool,
            tc.tile_pool(name="pw", bufs=3, space="PSUM") as pw_pool,
            tc.tile_pool(name="pr", bufs=1, space="PSUM") as pr_pool,
        ):
            a_t = a_pool.tile([128, NSTAGES, 8, BCHUNKS, 128], FP8)
            at_sb = at_pool.tile([128, MCHUNKS, 2, B], FP8)
            c_t = c_pool.tile([128, KCH, M], FP8)
            edid_t = ed_pool.tile([128, BCHUNKS + 4, 2 * M], FP8)
            ed_t = edid_t[:, 0:BCHUNKS]
            id_t = edid_t[:, BCHUNKS : BCHUNKS + 2]
            dconst_t = edid_t[:, BCHUNKS : BCHUNKS + 2, 0:M]
            mask_t = edid_t[:, BCHUNKS + 2 : BCHUNKS + 4]

            # --- DMA streams, one per queue, in consumption order -------
            # SP: edid (G1's first need) then the whole a stream.
            nc.sync.dma_start(out=edid_t[:], in_=edid[:])
            nc.sync.dma_start(out=a_t[:, 0, 0:2], in_=a[:, 0, 0:2])
            nc.sync.dma_start(out=a_t[:, 0, 2:8], in_=a[:, 0, 2:8])
            for j in range(1, NSTAGES):
                nc.sync.dma_start(out=a_t[:, j], in_=a[:, j])
            # ACT: c first half, then ats chunks 0..19 in 2-ch pieces.
            nc.scalar.dma_start(out=c_t[:, 0:16, :], in_=c[:, 0:16, :])
            for i in range(0, 20, 2):
                nc.scalar.dma_start(
                    out=at_sb[:, i : i + 2], in_=ats[:, i : i + 2]
                )
            # Pool (SWDGE): c second half, then ats chunks 20..31 in 2-ch
            # pieces (small, so update ops slot between transfers).
            nc.gpsimd.dma_start(out=c_t[:, 16:40, :], in_=c[:, 16:40, :])
            nc.gpsimd.dma_start(out=c_t[:, 40:64, :], in_=c[:, 40:64, :])

            psum_r0 = pr_pool.tile([128, 512], F32, name="psum_r0")
            psum_r1 = pr_pool.tile([128, 512], F32, name="psum_r1")
            psum_r = [psum_r0, psum_r1]
            # Wake the Activation engine early (first instruction pays the
            # ~1.4us act-table load) and absorb the c DMA waits on Pool.
            warm = tmp_pool.tile([128, 2, 128], F32)
            nc.scalar.copy(warm[:, 0, 0:1], edid_t[:, 0, 0:1])

            land = {}
            for nm in ("c0", "c1", "c2", "rt", "pw_act", "pw_dve"):
                land[nm] = land_pool.tile([128, 1], F32, name=f"land_{nm}")
            nc.gpsimd.tensor_copy(land["c0"][:], c_t[:, 0, 0:1])

            pw_cur = [None]
            pws_hist = {"act": [], "dve": []}

            def g1(ch, sub):
                k = 2 * ch + sub
                j, ub = k // 8, k % 8
                pw = pw_cur[0]
                for q in range(4):
                    nc.tensor.matmul(
                        pw[:, sub, :],
                        a_t[:, j, ub, 2 * q : 2 * q + 2, :],
                        ed_t[:, 2 * q : 2 * q + 2, :],
                        start=(q == 0),
                        stop=False,
                        perf_mode=DR,
                    )
                nc.tensor.matmul(
                    pw[:, sub, :],
                    dconst_t[:],
                    mask_t[:],
                    start=False,
                    stop=True,
                    perf_mode=DR,
                )

            def update(ch, cp, peng):
                # pw = [1-We | Wa/2];  C'/2 = (1 - We) * (C/2) + Wa/2
                # GPSIMD cannot read PSUM: ACT/DVE land pw into SBUF, then
                # Pool does both elementwise ops there.
                pw = pw_cur[0]
                pcopy = nc.scalar.copy if peng == "act" else nc.vector.tensor_copy
                pool_ = pwsa_pool if peng == "act" else pwsd_pool
                hist = pws_hist[peng]
                if len(hist) >= 3:
                    # absorb the pw_sb WAR (Pool ADD of the buf's previous
                    # user) on the copy engine so the pw-copy carries only
                    # its PE wait
                    pcopy(land["pw_" + peng][:], hist[-3][:, 0, 0:1])
                pw_sb = pool_.tile([128, 2, 2 * M], F32, name="pw_sb_" + peng)
                pcopy(pw_sb[:], pw[:])
                tmp2 = tmp_pool.tile([128, 2, 128], F32)
                nc.gpsimd.tensor_tensor(
                    tmp2[:],
                    pw_sb[:, :, 0:M],
                    c_t[:, 2 * ch : 2 * ch + 2, :],
                    MULT,
                )
                nc.gpsimd.tensor_add(cp[:], tmp2[:], pw_sb[:, :, M : 2 * M])
                hist.append(cp)

            g2_idx = [0]

            def g2(ch, cp):
                idx = g2_idx[0]
                g2_idx[0] += 1
                for jj in ((1, 0) if idx == MCHUNKS - 1 else (0, 1)):
                    nc.tensor.matmul(
                        psum_r[jj][:],
                        cp[:],
                        at_sb[:, ch, :, jj * 512 : (jj + 1) * 512],
                        start=(idx == 0),
                        stop=(idx == MCHUNKS - 1),
                        perf_mode=DR,
                    )

            G2_LAG = 4
            pending = []

            def flush_g2(pos):
                while pending and pending[0][0] <= pos - G2_LAG:
                    _, pc, pcp = pending.pop(0)
                    g2(pc, pcp)

            for ch in range(MCHUNKS):
                if ch == 8:
                    nc.gpsimd.tensor_copy(land["c1"][:], c_t[:, 16, 0:1])
                elif ch == 20:
                    nc.gpsimd.tensor_copy(land["c2"][:], c_t[:, 40, 0:1])
                if ch % 2 == 0 and 20 + ch // 2 < 32 and ch >= 8:
                    i = 20 + (ch - 8) // 2 * 2
                    if i < 32:
                        nc.gpsimd.dma_start(
                            out=at_sb[:, i : i + 2], in_=ats[:, i : i + 2]
                        )
                pw_cur[0] = pw_pool.tile([128, 2, 2 * M], F32, name="pw")
                g1(ch, 0)
                flush_g2(ch)
                g1(ch, 1)
                cp = cp_pool.tile([128, 2, M], FP8)
                update(ch, cp, "dve")
                pending.append((ch, ch, cp))

            for _, pc, pcp in pending:
                g2(pc, pcp)

            # Tail: each rt half is copied and stored by the same engine
            # over its own path (ACT HWDGE / DVE+Pool SWDGE) in parallel.
            rt_t = rt_pool.tile([128, B], BF16, name="rt_t")
            nc.scalar.copy(rt_t[:, 0:512], psum_r0[:])
            nc.scalar.dma_start(out=rt[:, 0:512], in_=rt_t[:, 0:512])
            nc.vector.tensor_copy(rt_t[:, 512:1024], psum_r1[:])
            nc.gpsimd.dma_start(out=rt[:, 512:1024], in_=rt_t[:, 512:1024])

    # Same-engine self-waits are always satisfied by in-order queue
    # completion; drop them. Anything still multi-wait is a structural bug.
    for inst in nc.inst_map.values():
        si = inst.sync_info
        if si and si.on_wait and len(si.on_wait) > 1:
            eng = str(inst.engine).split(".")[-1]
            kept = [w for w in si.on_wait if not w.ant_name.startswith(eng + "_")]
            assert len(kept) == 1, (
                f"{inst.name} on {eng} has waits {[w.ant_name for w in si.on_wait]}"
            )
            si.on_wait = kept

    return nc


def _prep_inputs(address, erase, add, content):
    f8 = ml_dtypes.float8_e4m3
    a_f8 = address.astype(f8)                                 # [1024, 65536]
    ed = np.concatenate([-erase, 0.5 * add], axis=1)          # [1024, 256] f32
    ed_hi = ed.astype(f8)
    ed_r = np.ascontiguousarray(
        ed_hi.reshape(BCHUNKS, 128, 2 * M).transpose(1, 0, 2)
    )                                                         # [128, 8, 256]
    c_bf = (0.5 * content).astype(f8)                         # [65536, 128]

    ident = np.zeros((128, 2, 256), dtype=f8)
    for i in range(2):
        for p in range(128):
            ident[p, i, i * 128 + p] = 1.0
    maskt = np.zeros((128, 2, 256), dtype=f8)
    maskt[:, :, 0:M] = 1.0
    edid = np.concatenate([ed_r, ident, maskt], axis=1)       # [128, 12, 256]

    in_maps = []
    for ci in range(NCORES):
        a_c = a_f8[:, ci * NS : (ci + 1) * NS]                # [1024, 8192]
        # a_r[p, j, ub, bc, u] = A[bc*128+p, j*1024+ub*128+u]
        a_r = np.ascontiguousarray(
            a_c.reshape(BCHUNKS, 128, NSTAGES, 8, 128).transpose(1, 2, 3, 0, 4)
        )                                                     # [128, 8, 8, 8, 128]
        # ats_r[p, ch, s, b] = A[b, ch*256 + s*128 + p]
        ats_r = np.ascontiguousarray(
            a_c.T.reshape(MCHUNKS, 2, 128, B).transpose(2, 0, 1, 3)
        )                                                     # [128, 32, 2, 1024]
        c_c = c_bf[ci * NS : (ci + 1) * NS, :]
        c_r = np.ascontiguousarray(
            c_c.reshape(KCH, 128, M).transpose(1, 0, 2)
        )                                                     # [128, 64, 128]
        in_maps.append({"a": a_r, "ats": ats_r, "c": c_r, "edid": edid})
    return in_maps


def kernel(address, erase, add, content, _trace=False, _result_box=None):
    if "nc" not in _compiled:
        _compiled["nc"] = _build_nc()
    nc = _compiled["nc"]

    in_maps = _prep_inputs(address, erase, add, content)
    res = run_bass_kernel_spmd(
        nc, in_maps, core_ids=list(range(NCORES)), trace=_trace
    )
    if _result_box is not None:
        _result_box.append(res)

    acc = np.zeros((M, B), dtype=np.float32)
    for r in res.results:
        acc += np.asarray(r["rt"], dtype=np.float32)
    return np.ascontiguousarray((2.0 * acc).T)
